# revision 1
# baseline (speedup 1.0000x reference)
"""BlockCirculantLinear kernel for 8x TRN2 NeuronCores.

Math: the reference's per-block circular correlation via FFT is exactly a
dense matmul out = (x * D) @ M where M[j*b+s, o*b+t] = W[o, j, (s-t) mod b].
D is folded into x on the host. The circulant blocks of M are never
materialized in DRAM: each on-chip M tile is fetched with an overlapping
-window DMA access pattern over wd = concat(W, W, axis=-1) ("window trick"):
with reversed tile columns t' = b-1-t,  M_block[s, t] = wd[o, j, 1 + s + t'],
so every SBUF row is a contiguous 512 B slice of wd. The column reversal is
undone on the host for free.

Batch is sharded across the 8 cores (data parallel, weights replicated).

Matmul dtype is float32r: fp32 storage, the PE truncates operands to the top
12 significand bits and streams at full rate (4x faster than fp32 mode, and
exact for operands with <=11 explicit mantissa bits). Measured end-to-end
relative error ~1.4e-4 (vs 2.6e-3 for bf16, 4e-7 for plain fp32 at 3.4x the
runtime). HW exec time ~519 us per core (TensorE active 473 us, 84% MFU).

Per-core device program (SPMD, same NEFF on all 8 cores):
  inputs : xT [128, 32, 1024] f32 ((x*D) shard, partition-major tiled)
           wd [32, 32, 256] f32 (doubled W rows)
  output : outT [4096, 1024] f32 (out shard, transposed, block-reversed)

  x is cached fully in SBUF (16 MB, 16 tiles, ACT HWDGE ring). M tiles stream
  through SBUF in [128, 8, 128] window chunks (SP HWDGE ring) so the first
  matmul starts ~15 us in. For each output block-row nt, psum[t'(128),
  m-chunk(512)] accumulates over the 32 k-tiles with lhsT = M-tile
  (stationary), rhs = x-tile (moving); 4 block-row groups are in flight
  across the 8 PSUM banks.
"""

import numpy as np

B_TOTAL = 8192
D_IN = 4096
D_OUT = 4096
BLK = 128
K_IN = D_IN // BLK    # 32
K_OUT = D_OUT // BLK  # 32
N_CORES = 8
B_SHARD = B_TOTAL // N_CORES  # 1024

P = 128
KO = D_IN // P                 # 32 k-tiles of 128
XC_SPLIT = 16                  # x-cache tiles (KO/XC_SPLIT k-tiles each)
KO_PER_XC = KO // XC_SPLIT
N_TILES = K_OUT                # 32 chunks of 128 output columns
MM_FREE = 512                  # moving free dim per matmul (one PSUM bank)
M_CHUNKS = B_SHARD // MM_FREE  # 2
WDL = 2 * BLK                  # doubled-W row length
MT_CHUNKS = 4                  # window-DMA chunks per M tile
KO_PER_MT = KO // MT_CHUNKS

_compiled = None


def _wd_window_ap(bass_mod, wd, nt):
    """Overlapping-window source AP into wd [K_OUT, K_IN, WDL] for output
    block-row nt: shape [128(s), K_IN(j), 128(t')], elem = wd[nt, j, 1+s+t']."""
    return bass_mod.AP(wd, (nt * K_IN) * WDL + 1, [[1, P], [WDL, K_IN], [1, BLK]])


def _build_module():
    import concourse.bass as bass
    import concourse.tile as tile
    from concourse import bacc, mybir

    nc = bacc.Bacc("TRN2", target_bir_lowering=False, debug=False)

    f32r = mybir.dt.float32r
    f32 = mybir.dt.float32

    xT = nc.dram_tensor("xT", [P, KO, B_SHARD], f32r, kind="ExternalInput")
    wd = nc.dram_tensor("wd", [K_OUT, K_IN, WDL], f32r, kind="ExternalInput")
    outT = nc.dram_tensor("outT", [D_OUT, B_SHARD], f32, kind="ExternalOutput")

    with tile.TileContext(nc) as tc:
        with (
            tc.tile_pool(name="xcache", bufs=1) as xpool,
            tc.tile_pool(name="mtiles", bufs=12) as mpool,
            tc.tile_pool(name="otiles", bufs=3) as opool,
            tc.tile_pool(name="psum", bufs=4, space="PSUM") as psum_pool,
        ):
            # x caches go on the ACT HWDGE ring; M-tile window loads use the
            # SP HWDGE ring — two parallel FIFOs, so neither queues behind
            # the other and the first matmul can start ~15 us in
            xcs = []
            for xi in range(XC_SPLIT):
                xc = xpool.tile([P, KO_PER_XC, B_SHARD], f32r, name=f"xc{xi}")
                nc.scalar.dma_start(
                    xc[:], xT[:, xi * KO_PER_XC : (xi + 1) * KO_PER_XC, :]
                )
                xcs.append(xc)

            for nt in range(N_TILES):
                mts = []
                for mi in range(MT_CHUNKS):
                    mt = mpool.tile(
                        [P, KO_PER_MT, BLK], f32r, tag="mt", name=f"mt_{nt}_{mi}"
                    )
                    src = _wd_window_ap(bass, wd, nt)
                    nc.sync.dma_start(
                        mt[:], src[:, mi * KO_PER_MT : (mi + 1) * KO_PER_MT, :]
                    )
                    mts.append(mt)
                psums = [
                    psum_pool.tile([P, MM_FREE], f32, tag=f"ps{i}", name=f"ps{i}_{nt}")
                    for i in range(M_CHUNKS)
                ]
                for ko in range(KO):
                    xc = xcs[ko // KO_PER_XC]
                    kk = ko % KO_PER_XC
                    mt = mts[ko // KO_PER_MT]
                    for mc in range(M_CHUNKS):
                        nc.tensor.matmul(
                            psums[mc][:],
                            lhsT=mt[:, ko % KO_PER_MT, :],
                            rhs=xc[:, kk, mc * MM_FREE : (mc + 1) * MM_FREE],
                            start=(ko == 0),
                            stop=(ko == KO - 1),
                        )
                ot = opool.tile([P, B_SHARD], f32, tag="ot", name=f"ot{nt}")
                for mc in range(M_CHUNKS):
                    nc.vector.tensor_copy(
                        ot[:, mc * MM_FREE : (mc + 1) * MM_FREE], psums[mc][:]
                    )
                nc.sync.dma_start(outT[nt * BLK : (nt + 1) * BLK, :], ot[:])

    nc.compile()
    return nc


def _get_module():
    global _compiled
    if _compiled is None:
        _compiled = _build_module()
    return _compiled


def kernel(x: np.ndarray, W: np.ndarray, D_bernoulli: np.ndarray) -> np.ndarray:
    from concourse.bass_utils import run_bass_kernel_spmd

    x = np.asarray(x, dtype=np.float32)
    W = np.asarray(W, dtype=np.float32)
    D = np.asarray(D_bernoulli, dtype=np.float32)

    xd = x * D[None, :]
    wd = np.ascontiguousarray(np.concatenate([W, W], axis=-1))  # [32, 32, 256]

    in_maps = []
    for c in range(N_CORES):
        xs = xd[c * B_SHARD : (c + 1) * B_SHARD].T          # [4096, 1024]
        # partition-major pre-tiling: [p, ko, m], 8KB-contiguous per p-chunk
        xs = np.ascontiguousarray(
            xs.reshape(KO, P, B_SHARD).transpose(1, 0, 2)
        )
        in_maps.append({"xT": xs, "wd": wd})

    nc = _get_module()
    res = run_bass_kernel_spmd(nc, in_maps, core_ids=list(range(N_CORES)))

    out = np.empty((B_TOTAL, D_OUT), dtype=np.float32)
    for c in range(N_CORES):
        oT = res.results[c]["outT"]                      # [4096, 1024]
        oT = oT.reshape(K_OUT, BLK, B_SHARD)[:, ::-1, :] # undo column reversal
        out[c * B_SHARD : (c + 1) * B_SHARD] = oT.reshape(D_OUT, B_SHARD).T
    return out



# revision 3
# speedup vs baseline: 7.0242x; 7.0242x over previous
"""BlockCirculantLinear kernel for 8x TRN2 NeuronCores — FFT-domain einsum.

Math: out = (x*D) @ M with M block-circulant (32x32 blocks of 128-circulants).
The reference computes it via per-block circular correlation in the FFT
domain; the dense-matmul formulation costs 2*B*4096^2 FLOPs, but the
frequency-domain einsum out_fft[b,o,f] = sum_j Xf[b,j,f] * conj(Wf)[o,j,f]
costs ~32x less. Host does the (cheap, O(B*d log b)) rfft/irfft and data
packing; the device does the einsum — the actual FLOPs — as bf16 matmuls.

Packing: rfft of a real 128-signal = 65 bins; bins 1..63 complex, bins
0/64 real. We pack exactly 128 real planes per block: R0..R63 and
I0..I63 with the I0 slot carrying R64. Planes are grouped 4 bins at a
time into 128-partition tiles (p = fi*32 + j), and the per-bin 32x32
complex multiply becomes 4 block-diagonal [128,128] real matmuls:
  psR[(fi,o),m] = A.XR + B.XI ;  psI[(fi,o),m] = C.XR + D.XI
with A=Re(V), B=-Im(V), C=Im(V), D=Re(V) (V = conj(rfft(W))) generically,
and the (g=0,fi=0) slot special-cased so psR0 = V0R.R0 (bin-0 real out)
and psI0 = V64R.R64 (bin-64 real out). Host irfft undoes the packing.

Batch is data-parallel across 8 cores (1024 samples each). Per-core
device program: 16 groups x 8 matmuls [128,128]x[128,512] bf16 -> f32
PSUM; psR evacuated by VectorE, psI by ScalarE (both cast to bf16);
I/O = 8MB in + 8MB out + 2MB weights, streamed on the two HWDGE rings.

Measured end-to-end relative error ~3e-3 (bf16 operand/output rounding;
fp8 inputs were tested and fail the 2e-2 gate at 2.7e-2).
"""

import numpy as np
import ml_dtypes

B_TOTAL = 8192
D_IN = 4096
D_OUT = 4096
BLK = 128
K_IN = D_IN // BLK    # 32
K_OUT = D_OUT // BLK  # 32
N_CORES = 8
B_SHARD = B_TOTAL // N_CORES  # 1024
NB = BLK // 2 + 1     # 65 rfft bins
G = 16                # groups of 4 packed bins (64 plane-pairs)
MM_FREE = 512         # moving free dim per matmul (one PSUM bank)

_compiled = None


def _build_module():
    import concourse.bass as bass
    import concourse.tile as tile
    from concourse import bacc, mybir

    nc = bacc.Bacc("TRN2", target_bir_lowering=False, debug=False)

    bf = mybir.dt.bfloat16
    f32 = mybir.dt.float32

    # xf[c, g, p, m]: c=0 XR planes, c=1 XI planes; p = fi*32+j
    xf = nc.dram_tensor("xf", [2, G, 128, B_SHARD], bf, kind="ExternalInput")
    # wt[p, g, wk, q]: block-diag lhsT matrices, wk in (A, B, C, D)
    wt = nc.dram_tensor("wt", [128, G, 4, 128], bf, kind="ExternalInput")
    # yf[g, c, p, m]: c=0 psR, c=1 psI; p = fi*32+o
    yf = nc.dram_tensor("yf", [G, 2, 128, B_SHARD], bf, kind="ExternalOutput")

    with tile.TileContext(nc) as tc:
        with (
            tc.tile_pool(name="wts", bufs=1) as wpool,
            tc.tile_pool(name="xin", bufs=8) as xpool,
            tc.tile_pool(name="yout", bufs=3) as opool,
            tc.tile_pool(name="psum", bufs=2, space="PSUM") as ppool,
        ):
            wtg = []
            for g in range(G):
                w = wpool.tile([128, 4, 128], bf, name=f"wt{g}")
                nc.sync.dma_start(
                    w[:], bass.AP(wt, g * 4 * 128, [[G * 4 * 128, 128], [128, 4], [1, 128]])
                )
                wtg.append(w)

            for g in range(G):
                xr = xpool.tile([128, B_SHARD], bf, tag="xr", name=f"xr{g}")
                nc.scalar.dma_start(
                    xr[:], bass.AP(xf, g * 128 * B_SHARD, [[B_SHARD, 128], [1, B_SHARD]])
                )
                xi = xpool.tile([128, B_SHARD], bf, tag="xi", name=f"xi{g}")
                nc.scalar.dma_start(
                    xi[:],
                    bass.AP(xf, (G + g) * 128 * B_SHARD, [[B_SHARD, 128], [1, B_SHARD]]),
                )

                psR = ppool.tile([128, B_SHARD], f32, tag="psR", name=f"psR{g}")
                psI = ppool.tile([128, B_SHARD], f32, tag="psI", name=f"psI{g}")
                w = wtg[g]
                for mc in range(B_SHARD // MM_FREE):
                    s = slice(mc * MM_FREE, (mc + 1) * MM_FREE)
                    nc.tensor.matmul(psR[:, s], lhsT=w[:, 0, :], rhs=xr[:, s], start=True, stop=False)
                    nc.tensor.matmul(psI[:, s], lhsT=w[:, 2, :], rhs=xr[:, s], start=True, stop=False)
                    nc.tensor.matmul(psR[:, s], lhsT=w[:, 1, :], rhs=xi[:, s], start=False, stop=True)
                    nc.tensor.matmul(psI[:, s], lhsT=w[:, 3, :], rhs=xi[:, s], start=False, stop=True)

                ot = opool.tile([128, 2, B_SHARD], bf, tag="ot", name=f"ot{g}")
                nc.vector.tensor_copy(ot[:, 0, :], psR[:])
                nc.scalar.copy(ot[:, 1, :], psI[:])
                nc.sync.dma_start(
                    bass.AP(
                        yf,
                        g * 2 * 128 * B_SHARD,
                        [[B_SHARD, 128], [128 * B_SHARD, 2], [1, B_SHARD]],
                    ),
                    ot[:],
                )

    nc.compile()
    return nc


def _get_module():
    global _compiled
    if _compiled is None:
        _compiled = _build_module()
    return _compiled


def kernel(x: np.ndarray, W: np.ndarray, D_bernoulli: np.ndarray) -> np.ndarray:
    from concourse.bass_utils import run_bass_kernel_spmd

    bf16 = ml_dtypes.bfloat16
    x = np.asarray(x, dtype=np.float32)
    W = np.asarray(W, dtype=np.float32)
    D = np.asarray(D_bernoulli, dtype=np.float32)

    # --- host: forward rfft of (x*D) blocks, pack 64 plane-pair groups ---
    xd = (x * D[None, :]).reshape(B_TOTAL, K_IN, BLK)
    Xf = np.fft.rfft(xd, axis=-1)                 # [B, 32, 65]
    Xr = np.ascontiguousarray(Xf.real.transpose(2, 1, 0))  # [65, 32, B]
    Xi = np.ascontiguousarray(Xf.imag.transpose(2, 1, 0))
    XR = Xr[:64]                                  # [64, 32, B]
    XI = Xi[:64].copy()
    XI[0] = Xr[64]                                # R64 rides in the I0 slot
    xr_g = XR.reshape(G, 128, B_TOTAL)            # p = fi*32+j
    xi_g = XI.reshape(G, 128, B_TOTAL)

    # --- host: weights -> block-diagonal lhsT matrices [p, G, 4, q] ---
    Vf = np.conj(np.fft.rfft(W, axis=-1))         # [o, j, 65]
    VR = Vf.real.transpose(2, 1, 0)               # [65, j, o]
    VI = Vf.imag.transpose(2, 1, 0)
    A = VR[:64].copy()
    Bm = (-VI[:64]).copy()
    C = VI[:64].copy()
    Dm = VR[:64].copy()
    Bm[0] = 0.0                                   # bin-0/64 real-only slots
    C[0] = 0.0
    Dm[0] = VR[64]
    Wk = np.zeros((G, 4, 128, 128), dtype=np.float32)
    for wk, M in enumerate((A, Bm, C, Dm)):
        Mr = M.reshape(G, 4, K_IN, K_OUT)
        for fi in range(4):
            Wk[:, wk, fi * 32 : (fi + 1) * 32, fi * 32 : (fi + 1) * 32] = Mr[:, fi]
    wt_host = np.ascontiguousarray(Wk.transpose(2, 0, 1, 3)).astype(bf16)

    in_maps = []
    for c in range(N_CORES):
        sl = slice(c * B_SHARD, (c + 1) * B_SHARD)
        xfc = np.empty((2, G, 128, B_SHARD), dtype=bf16)
        xfc[0] = xr_g[:, :, sl]
        xfc[1] = xi_g[:, :, sl]
        in_maps.append({"xf": xfc, "wt": wt_host})

    nc = _get_module()
    res = run_bass_kernel_spmd(nc, in_maps, core_ids=list(range(N_CORES)))

    # --- host: unpack spectra, irfft, reassemble ---
    out = np.empty((B_TOTAL, D_OUT), dtype=np.float32)
    for c in range(N_CORES):
        y = np.asarray(res.results[c]["yf"], dtype=np.float32)  # [G, 2, 128, m]
        psR = y[:, 0].reshape(64, K_OUT, B_SHARD)
        psI = y[:, 1].reshape(64, K_OUT, B_SHARD)
        Yf = np.zeros((B_SHARD, K_OUT, NB), dtype=np.complex64)
        Yf[:, :, :64] = (psR + 1j * psI).transpose(2, 1, 0)
        Yf[:, :, 0] = psR[0].T
        Yf[:, :, 64] = psI[0].T
        ob = np.fft.irfft(Yf, n=BLK, axis=-1)     # [m, 32, 128]
        out[c * B_SHARD : (c + 1) * B_SHARD] = ob.reshape(B_SHARD, D_OUT)
    return out


# revision 5
# speedup vs baseline: 8.1041x; 1.1537x over previous
"""BlockCirculantLinear kernel for 8x TRN2 NeuronCores — FFT-domain einsum.

Math: out = (x*D) @ M with M block-circulant (32x32 blocks of 128-circulants).
The reference computes per-block circular correlation in the FFT domain; a
dense matmul costs 2*B*4096^2 FLOPs but the frequency-domain einsum
out_fft[b,o,f] = sum_j Xf[b,j,f] * conj(Wf)[o,j,f] costs ~32x less. Host
does the cheap O(B d log b) rfft/irfft + packing; the device does the
einsum — where the FLOPs are — as bf16 matmuls.

Packing: rfft of a real 128-signal = 65 bins; bins 1..63 complex, 0/64
real. Exactly 128 real planes per block: R0..R63, I0..I63 with the I0
slot carrying R64. Planes are grouped 4 bins per 128-partition tile
(p = fi*32 + j) and the per-bin 32x32 complex multiply becomes 4 real
matmuls psR = A.XR + B.XI, psI = C.XR + D.XI with A=Re(V), B=-Im(V),
C=Im(V), D=Re(V), V = conj(rfft(W)); the (g=0,fi=0) slot is special-
cased (B=C=0, D=Re(V64)) so psR0/psI0 carry the two real bins. The j-
contraction is only 32 deep, so the 4 bins of a group run as concurrent
32x32 quadrant matmuls via tile_position=(32fi,32fi) — weights stay
dense (0.5MB, not 2MB block-diagonal).

Batch is data-parallel across 8 cores (1024 samples each). Per-core:
16 groups x 8 accumulation steps of 4 quadrant matmuls [32,32]x[32,512]
bf16 -> f32 PSUM; psR evacuated by VectorE, psI by ScalarE, cast bf16.
I/O: 8MB in + 8MB out + 0.5MB weights, 4KB/partition DMA rows; inputs+
weights stream on the SP HWDGE ring, outputs on the ACT ring (so the
ACT-table load doesn't gate the first input). ~300-380 GB/s on the wire
(per-NC HBM limit) is the bottleneck; PE ~27us warm.

Measured end-to-end relative error ~3e-3 (bf16 rounding; fp8 inputs
fail the 2e-2 gate at 2.7e-2).
"""

import numpy as np
import ml_dtypes

B_TOTAL = 8192
D_IN = 4096
D_OUT = 4096
BLK = 128
K_IN = D_IN // BLK    # 32
K_OUT = D_OUT // BLK  # 32
N_CORES = 8
B_SHARD = B_TOTAL // N_CORES  # 1024
NB = BLK // 2 + 1     # 65 rfft bins
G = 16                # groups of 4 packed bins (64 plane-pairs)
MM_FREE = 512         # moving free dim per matmul (one PSUM bank)

_compiled = None


def _build_module():
    import concourse.bass as bass
    import concourse.tile as tile
    from concourse import bacc, mybir

    nc = bacc.Bacc("TRN2", target_bir_lowering=False, debug=False)

    bf = mybir.dt.bfloat16
    f32 = mybir.dt.float32

    # xf[g, p, c, m]: c=0 XR plane, c=1 XI plane; p = fi*32+j; 4KB rows
    xf = nc.dram_tensor("xf", [G, 128, 2, B_SHARD], bf, kind="ExternalInput")
    # wt[p, g, wk, q]: dense per-quadrant lhsT blocks, wk in (A, B, C, D)
    wt = nc.dram_tensor("wt", [128, G, 4, 32], bf, kind="ExternalInput")
    # yf[g, p, c, m]: c=0 psR, c=1 psI; p = fi*32+o; 4KB rows
    yf = nc.dram_tensor("yf", [G, 128, 2, B_SHARD], bf, kind="ExternalOutput")

    with tile.TileContext(nc) as tc:
        with (
            tc.tile_pool(name="wts", bufs=1) as wpool,
            tc.tile_pool(name="xin", bufs=12) as xpool,
            tc.tile_pool(name="yout", bufs=4) as opool,
            tc.tile_pool(name="psum", bufs=2, space="PSUM") as ppool,
        ):
            xts = []
            for g in range(G):
                xt = xpool.tile([128, 2, B_SHARD], bf, tag="xt", name=f"xt{g}")
                nc.sync.dma_start(
                    xt[:],
                    bass.AP(
                        xf,
                        g * 128 * 2 * B_SHARD,
                        [[2 * B_SHARD, 128], [B_SHARD, 2], [1, B_SHARD]],
                    ),
                )
                xts.append(xt)
                if g == 0:
                    w = wpool.tile([128, G, 4, 32], bf, name="wt")
                    nc.sync.dma_start(w[:], wt[:])

            for g in range(G):
                xt = xts[g]
                psR = ppool.tile([128, B_SHARD], f32, tag="psR", name=f"psR{g}")
                psI = ppool.tile([128, B_SHARD], f32, tag="psI", name=f"psI{g}")
                for mc in range(B_SHARD // MM_FREE):
                    s = slice(mc * MM_FREE, (mc + 1) * MM_FREE)
                    # (wk, c, dst, start, stop): psR = A.XR + B.XI ; psI = C.XR + D.XI
                    for wk, c, ps, st, sp in (
                        (0, 0, psR, True, False),
                        (2, 0, psI, True, False),
                        (1, 1, psR, False, True),
                        (3, 1, psI, False, True),
                    ):
                        for fi in range(4):
                            q = slice(fi * 32, (fi + 1) * 32)
                            nc.tensor.matmul(
                                ps[q, s],
                                lhsT=w[q, g, wk, :],
                                rhs=xt[q, c, s],
                                start=st,
                                stop=sp,
                                tile_position=(fi * 32, fi * 32),
                            )

                ot = opool.tile([128, 2, B_SHARD], bf, tag="ot", name=f"ot{g}")
                nc.vector.tensor_copy(ot[:, 0, :], psR[:])
                nc.scalar.copy(ot[:, 1, :], psI[:])
                nc.scalar.dma_start(
                    bass.AP(
                        yf,
                        g * 128 * 2 * B_SHARD,
                        [[2 * B_SHARD, 128], [B_SHARD, 2], [1, B_SHARD]],
                    ),
                    ot[:],
                )

    nc.compile()
    return nc


def _get_module():
    global _compiled
    if _compiled is None:
        _compiled = _build_module()
    return _compiled


def kernel(x: np.ndarray, W: np.ndarray, D_bernoulli: np.ndarray) -> np.ndarray:
    from concourse.bass_utils import run_bass_kernel_spmd

    bf16 = ml_dtypes.bfloat16
    x = np.asarray(x, dtype=np.float32)
    W = np.asarray(W, dtype=np.float32)
    D = np.asarray(D_bernoulli, dtype=np.float32)

    # --- host: forward rfft of (x*D) blocks, pack 64 plane-pair groups ---
    xd = (x * D[None, :]).reshape(B_TOTAL, K_IN, BLK)
    Xf = np.fft.rfft(xd, axis=-1)                 # [B, 32, 65]
    Xr = np.ascontiguousarray(Xf.real.transpose(2, 1, 0))  # [65, 32, B]
    Xi = np.ascontiguousarray(Xf.imag.transpose(2, 1, 0))
    XR = Xr[:64]                                  # [64, 32, B]
    XI = Xi[:64].copy()
    XI[0] = Xr[64]                                # R64 rides in the I0 slot
    # xf_all[g, p, c, m_global]
    xf_all = np.empty((G, 128, 2, B_TOTAL), dtype=bf16)
    xf_all[:, :, 0, :] = XR.reshape(G, 128, B_TOTAL)
    xf_all[:, :, 1, :] = XI.reshape(G, 128, B_TOTAL)

    # --- host: weights -> dense quadrant lhsT blocks [p, G, wk, 32] ---
    Vf = np.conj(np.fft.rfft(W, axis=-1))         # [o, j, 65]
    VR = Vf.real.transpose(2, 1, 0)               # [65, j, o]
    VI = Vf.imag.transpose(2, 1, 0)
    A = VR[:64].copy()
    Bm = (-VI[:64]).copy()
    C = VI[:64].copy()
    Dm = VR[:64].copy()
    Bm[0] = 0.0                                   # bin-0/64 real-only slots
    C[0] = 0.0
    Dm[0] = VR[64]
    Wd = np.stack((A, Bm, C, Dm), axis=1)         # [64, 4, j32, o32]
    # -> [p = fi*32+j, g, wk, o]
    wt_host = np.ascontiguousarray(
        Wd.reshape(G, 4, 4, K_IN, K_OUT).transpose(1, 3, 0, 2, 4).reshape(128, G, 4, K_OUT)
    ).astype(bf16)

    in_maps = []
    for c in range(N_CORES):
        sl = slice(c * B_SHARD, (c + 1) * B_SHARD)
        in_maps.append({"xf": np.ascontiguousarray(xf_all[:, :, :, sl]), "wt": wt_host})

    nc = _get_module()
    res = run_bass_kernel_spmd(nc, in_maps, core_ids=list(range(N_CORES)))

    # --- host: unpack spectra, irfft, reassemble ---
    out = np.empty((B_TOTAL, D_OUT), dtype=np.float32)
    for c in range(N_CORES):
        y = np.asarray(res.results[c]["yf"], dtype=np.float32)  # [G, 128, 2, m]
        yr = y.reshape(G, 4, K_OUT, 2, B_SHARD).reshape(64, K_OUT, 2, B_SHARD)
        psR = yr[:, :, 0, :]                      # [64, o, m]
        psI = yr[:, :, 1, :]
        Yf = np.zeros((B_SHARD, K_OUT, NB), dtype=np.complex64)
        Yf[:, :, :64] = (psR + 1j * psI).transpose(2, 1, 0)
        Yf[:, :, 0] = psR[0].T
        Yf[:, :, 64] = psI[0].T
        ob = np.fft.irfft(Yf, n=BLK, axis=-1)     # [m, 32, 128]
        out[c * B_SHARD : (c + 1) * B_SHARD] = ob.reshape(B_SHARD, D_OUT)
    return out
